# revision 1
# baseline (speedup 1.0000x reference)
"""Trainium2 kernel for nn_HadamardRotation: y = x @ H, H = 4096x4096 Walsh-Hadamard.

Strategy
--------
H4096 = H64 (x) H64 (Kronecker). Writing d = 64*hi + lo, e = 64*hi' + lo':

    y[r, e] = sum_{hi,lo} H64[lo,lo'] * H64[hi,hi'] * x[r, d]

Two matmul stages with 128-wide contraction (block-diagonal I2 (x) H64
weights), separated by an on-chip "corner turn" (SBUF->SBUF DMA partition
shuffle), all operating in the transposed domain (d on partitions, rows on
the free axis).

The corner turn dominates; it is tuned for the DMA engines' per-descriptor
cost: the whole per-core row range (L=2048) is kept in one SBUF-resident
intermediate so every turn descriptor is a full 4KB line, and the 32 turn
DMAs are spread over multiple queues so their descriptors hit all 16 DMA
engines. Input and output DRAM tensors exactly mirror the SBUF tiles
(contiguous 4-8KB per partition per DMA).

FLOPs: 2 * 128/4096 of the naive matmul = 16x reduction.

Data parallel over 8 cores: rows sharded 16384 -> 8 x 2048, weights
replicated. Host does the layout permutes / final f32 cast (not timed).

Per-core layouts (R = 2048 rows = L, N = 512 matmul slab):
  xt DRAM in  [16, 128, IB*L]: xt[g, q, j*L+rr] = x[rr, 128*(IB*g+j)+q]
  B1 (128,128): B1[64*mu+lo, 32*(2nu+mu)+c] = H64[lo, 2c+nu]
  B2 (128,128): B2[64*nu+32*mu+a, 2*hi'+nu] = H64[2*a+mu, hi']
  stage A (chunk a): u[m, a, rr] = sum_k B1[k, m] xg[k, j, rr]
      => u[32*(2nu+mu) + c, a] holds (hi = 2a+mu, lo' = 2c+nu)
  corner turn (chunk c): vc[32t+a, rr] = u[32t+c, a, rr]
  stage B (chunk c): yb[m2, rr] = sum_q B2[q, m2] vc[q, rr]
      => yb[2*hi'+nu] = y[rr, 64*hi' + 2*c + nu]  (bf16)
  Y DRAM out [32/OB, 128, OB*L]: mirrors the SBUF yb tiles; host
     unscrambles + casts to f32.
"""

import math
import numpy as np
import ml_dtypes

import concourse.bass as bass
import concourse.mybir as mybir
import concourse.tile as tile
from concourse import bacc
from concourse.bass_utils import run_bass_kernel_spmd

N_CORES = 8
DIM = 4096
R_TOTAL = 4 * 4096          # rows after flattening (4, 4096, DIM)
R = R_TOTAL // N_CORES      # rows per core
L = R                       # all rows resident: 4KB turn descriptors
N = 512                     # matmul free-dim slab (one PSUM bank of fp32)
TS = L // N                 # matmul slabs per chunk
MODE = "bf16"               # storage dtype for x/intermediate/output

# tuning knobs
CFG = dict(
    IB=2,              # chunks per input DMA / xg tile
    OB=2,              # chunks per output DMA / yb tile
    OB2=2,             # chunks per output DMA in the deferred ts=0 pass
    ucopy_engs="vector,scalar,scalar,scalar",  # stage-A psum->sbuf copies
    ycopy_engs="scalar,scalar,scalar,vector",  # stage-B psum->sbuf copies
    y2copy_engs="vector,scalar",  # deferred-pass copies
    in_engs="sync",
    out_engs="sync",
    out2_engs="gpsimd",
    turn_engs="gpsimd,sync",     # queues that never carry blocked out-DMAs
    W_DVE=256,         # leading r-window turned on the DVE stream unit
                       # (takes 2x its bytes off the saturated DMA fabric;
                       # sub-slab widths split the ts=0 matmul in two)
    turn_splits=4,     # stream-transpose instrs (finer DVE interleave)
    xbufs=3, vbufs=6, ybufs=3, psA=4, psB=4,
)


def _walsh_hadamard64():
    h = np.array([[1.0]], dtype=np.float64)
    while h.shape[0] < 64:
        h = np.block([[h, h], [h, -h]]) / math.sqrt(2.0)
    return h.astype(np.float32)


def _build_weights(H64):
    # B1[64*mu+lo, 32*(2*nu+mu')+c] = H64[lo, 2c+nu] if mu'==mu else 0
    B1 = np.zeros((128, 128), dtype=np.float32)
    b1v = B1.reshape(2, 64, 2, 2, 32)       # [mu, lo, nu, mu', c]
    for mu in range(2):
        for nu in range(2):
            b1v[mu, :, nu, mu, :] = H64[:, nu::2]
    # B2[64*nu+32*mu+a, 2*hi'+nu'] = H64[2a+mu, hi'] if nu'==nu else 0
    B2 = np.zeros((128, 128), dtype=np.float32)
    b2v = B2.reshape(2, 2, 32, 64, 2)       # [nu, mu, a, hi', nu']
    for nu in range(2):
        for mu in range(2):
            b2v[nu, mu, :, :, nu] = H64[mu::2, :]
    return B1, B2


_NC_CACHE = {}


def _build_bass(cfg=None):
    cfg = dict(CFG, **(cfg or {}))
    key = tuple(sorted(cfg.items()))
    if key in _NC_CACHE:
        return _NC_CACHE[key]

    f32 = mybir.dt.float32
    bf16 = mybir.dt.bfloat16

    IB, OB, OB2 = cfg["IB"], cfg["OB"], cfg["OB2"]
    NG = 32 // IB               # input chunk groups
    NCB = 32 // OB              # output chunk batches (main, ts>=1 slabs)
    NCB2 = 32 // OB2            # output chunk batches (deferred ts=0 pass)
    WD = cfg["W_DVE"]
    LM = L - WD                 # r-range covered by the main (DMA-turn) pass
    TSM = LM // N               # slabs in the main pass

    nc = bacc.Bacc("TRN2", target_bir_lowering=False, debug=False,
                   num_devices=N_CORES)
    LH = L // 2                 # r-half loaded per input DMA (ts-pair-major)
    xt_d = nc.dram_tensor("xt", [2 * NG, 128, IB * LH], bf16,
                          kind="ExternalInput")
    B1_d = nc.dram_tensor("B1", [128, 128], bf16, kind="ExternalInput")
    B2_d = nc.dram_tensor("B2", [128, 128], bf16, kind="ExternalInput")
    Y_d = nc.dram_tensor("Y", [NCB, 128, OB * L], bf16, kind="ExternalOutput")

    with tile.TileContext(nc) as tc:
        with (
            tc.tile_pool(name="wpool", bufs=1) as wpool,
            tc.tile_pool(name="xpool", bufs=cfg["xbufs"]) as xpool,
            tc.tile_pool(name="upool", bufs=1) as upool,
            tc.tile_pool(name="vpool", bufs=cfg["vbufs"]) as vpool,
            tc.tile_pool(name="vdpool", bufs=1) as vdpool,
            tc.tile_pool(name="ypool", bufs=cfg["ybufs"]) as ypool,
            tc.tile_pool(name="psA", bufs=cfg["psA"], space="PSUM") as psA,
            tc.tile_pool(name="psB", bufs=cfg["psB"], space="PSUM") as psB,
        ):
            B1_sb = wpool.tile([128, 128], bf16)
            nc.sync.dma_start(B1_sb[:], B1_d[:])
            B2_sb = wpool.tile([128, 128], bf16)
            nc.sync.dma_start(B2_sb[:], B2_d[:])


            def eng_list(names):
                return [getattr(nc, nm.strip()) for nm in names.split(",")]

            ucopy_engs = eng_list(cfg["ucopy_engs"])
            ycopy_engs = eng_list(cfg["ycopy_engs"])
            y2copy_engs = eng_list(cfg["y2copy_engs"])
            turn_engs = eng_list(cfg["turn_engs"])
            out_engs = eng_list(cfg["out_engs"])
            out2_engs = eng_list(cfg["out2_engs"])
            in_engs = eng_list(cfg["in_engs"])

            def copy(engs, i, dst, src):
                e = engs[i % len(engs)]
                if e is nc.scalar:
                    nc.scalar.copy(dst, src)
                else:
                    e.tensor_copy(dst, src)

            u_all = upool.tile([128, 32, L], bf16)
            ut = u_all.tensor
            PU = u_all.ap[0][0]  # partition stride in elements

            TSPL = cfg["turn_splits"]
            v_dve = None
            if WD:
                v_dve = vdpool.tile([128, 32, WD], bf16, name="v_dve")

            def emit_stream_turn():
                # v[32t+a, c, rr] = u[32t+c, a, rr] via DVE 32x32 stream
                # transposes over the leading W_DVE r-window, all chunks.
                vt = v_dve.tensor
                PV = v_dve.ap[0][0]
                WS = WD // TSPL
                for s in range(TSPL):
                    in_ap = bass.AP(ut, s * WS,
                                    [[PU, 128], [1, WS], [L, 32]])
                    out_ap = bass.AP(vt, s * WS,
                                     [[PV, 128], [1, WS], [WD, 32]])
                    nc.vector.transpose(out_ap, in_ap)

            # stage A: r-half-major so the DVE stream turn (which only needs
            # the first slab) can start at mid-phase
            for th in range(2):
                for g in range(NG):
                    xg = xpool.tile([128, IB, LH], bf16)
                    gg = th * NG + g
                    in_engs[gg % len(in_engs)].dma_start(xg[:], xt_d[gg, :, :])
                    for j in range(IB):
                        a = IB * g + j
                        for tl in range(TS // 2):
                            ts = th * (TS // 2) + tl
                            pu = psA.tile([128, N], f32)
                            nc.tensor.matmul(pu[:], B1_sb[:],
                                             xg[:, j, tl * N:(tl + 1) * N],
                                             start=True, stop=True)
                            copy(ucopy_engs, a * TS + ts,
                                 u_all[:, a, ts * N:(ts + 1) * N], pu[:])
                if th == 0 and WD:
                    emit_stream_turn()

            # corner turn part 2 (per-chunk DMA of the tail r-window) + stage B.
            # Pure-DMA slabs are emitted FIRST per chunk so the PE never
            # blocks waiting for the stream transposes to finish. The slab
            # containing the v_dve/vc boundary is computed as two half-width
            # matmuls into disjoint columns of one PSUM bank.
            for cb in range(NCB):
                yb = ypool.tile([128, OB, L], bf16)
                for j in range(OB):
                    c = cb * OB + j
                    vc = vpool.tile([128, LM], bf16)
                    in_ap = bass.AP(ut, c * PU + WD,
                                    [[32 * PU, 4], [L, 32], [1, LM]])
                    turn_engs[c % len(turn_engs)].dma_start(vc[:], in_ap)
                    for ts in list(range(1, TS)) + [0]:
                        py = psB.tile([128, N], f32)
                        if ts == 0 and WD:
                            nc.tensor.matmul(py[:, :WD], B2_sb[:],
                                             v_dve[:, c, :],
                                             start=True, stop=True)
                            nc.tensor.matmul(py[:, WD:], B2_sb[:],
                                             vc[:, :N - WD],
                                             start=True, stop=True)
                        else:
                            lo = ts * N - WD
                            nc.tensor.matmul(py[:], B2_sb[:],
                                             vc[:, lo:lo + N],
                                             start=True, stop=True)
                        copy(ycopy_engs, c * TS + ts,
                             yb[:, j, ts * N:(ts + 1) * N], py[:])
                out_engs[cb % len(out_engs)].dma_start(Y_d[cb, :, :], yb[:])

    nc.compile()
    _NC_CACHE[key] = nc
    return nc


def _prep_inputs(x, H, cfg=None):
    cfg = dict(CFG, **(cfg or {}))
    IB = cfg["IB"]
    NG = 32 // IB
    H64 = (np.asarray(H, dtype=np.float32)[::64, ::64] * 8.0).astype(np.float32)
    B1, B2 = _build_weights(H64)
    B1 = B1.astype(ml_dtypes.bfloat16)
    B2 = B2.astype(ml_dtypes.bfloat16)
    xf = np.asarray(x, dtype=np.float32).reshape(R_TOTAL, DIM)
    in_maps = []
    LH = L // 2
    for i in range(N_CORES):
        shard = xf[i * R:(i + 1) * R]                     # (R, DIM)
        # [(th, rr), a, q] -> [th, g, q, j, rr]
        xt = shard.reshape(2, LH, 32, 128).transpose(0, 2, 3, 1)  # [th,a,q,rr]
        xt = xt.reshape(2, NG, IB, 128, LH).transpose(0, 1, 3, 2, 4)
        xt = np.ascontiguousarray(xt, dtype=ml_dtypes.bfloat16)
        xt = xt.reshape(2 * NG, 128, IB * LH)
        in_maps.append({"xt": xt, "B1": B1, "B2": B2})
    return in_maps


def _unscramble(results, cfg=None):
    cfg = dict(CFG, **(cfg or {}))
    OB, OB2, WD = cfg["OB"], cfg["OB2"], cfg["W_DVE"]
    NCB, NCB2 = 32 // OB, 32 // OB2
    LM = L - WD
    outs = []
    for i in range(N_CORES):
        # [cb, (hi', nu), j, rr] -> [rr, hi', (cb, j, nu)]
        Y = results[i]["Y"]      # [NCB, 128, OB*L] bf16
        y = np.asarray(Y, dtype=np.float32).reshape(NCB, 64, 2, OB, L)
        y = y.transpose(4, 1, 0, 3, 2).reshape(R, DIM)
        outs.append(y)
    return np.concatenate(outs, axis=0).reshape(4, 4096, DIM).astype(np.float32)


def kernel(x, H, _trace=False, _cfg=None):
    nc = _build_bass(_cfg)
    in_maps = _prep_inputs(x, H, _cfg)
    res = run_bass_kernel_spmd(nc, in_maps, core_ids=list(range(N_CORES)),
                               trace=_trace)
    out = _unscramble(res.results, _cfg)
    if _trace:
        return out, res
    return out



# revision 4
# speedup vs baseline: 2.7836x; 2.7836x over previous
"""Trainium2 kernel for nn_HadamardRotation: y = x @ H, H = 4096x4096 Walsh-Hadamard.

Strategy (v2: turn-free, data-stationary stage A)
-------------------------------------------------
H4096 = H128 (x) H32 (Kronecker over bit positions, d = 32*P + A):

    y[r, 32*EP + EA] = sum_{P,A} x[r, 32*P + A] * H128[P,EP] * H32[A,EA] / 64

Stage A loads the DATA as the stationary operand (lhsT), so the output
partition axis becomes the (row, A) free-chunk of x -- i.e. the stage-B
contraction bits (A) land on partitions with NO corner turn:

  xt DRAM [128, R*32] bf16:  xt[P, 32*r + A] = x[r, 32*P + A]   (host permute)
  MM_A (per 4 rows):  lhsT = xt chunk [128=(P), 128=(rl,A)], rhs = W1=H128/8
                      -> PSUM [(rl,A), EP];  4 chunks fill one [128,512] bank
  copy PSUM->SBUF bf16 (rotating scalar/vector/gpsimd)
  MM_B (per 16 rows): lhsT = W2 = I4 (x) H32/8 (contracts A, passes rl),
                      rhs = u [128,512] -> PSUM [(rl,EA), (j,EP)] = y
  copy -> staging, large contiguous out-DMAs.

DMA traffic is only 16.8MB in + 16.8MB out per core (HBM floor ~94us);
no SBUF->SBUF turn traffic at all.  Host does layout permutes + casts
(not timed), weights are exact (+-1/8) in bf16.

Data parallel over 8 cores: rows sharded 16384 -> 8 x 2048.
"""

import math
import numpy as np
import ml_dtypes

import concourse.bass as bass
import concourse.mybir as mybir
import concourse.tile as tile
from concourse import bacc
from concourse.bass_utils import run_bass_kernel_spmd

N_CORES = 8
DIM = 4096
R_TOTAL = 4 * 4096
R = R_TOTAL // N_CORES      # rows per core (2048)
NG = R // 16                # 16-row groups per core (128)
FREE = R * 32               # free extent of xt / Y (65536)
MODE = "bf16"

CFG = dict(
    skew=2,                       # groups of MM_A emitted ahead of MM_B
    in_blocks=[4, 4, 4, 4] + [16] * 7,    # groups per input DMA
    out_blocks=[16] * 7 + [4, 4, 4, 4],   # groups per output DMA
    ucopy_engs="scalar",
    ycopy_engs="vector",
    in_engs="sync",
    out_engs="gpsimd",
    xbufs=3, ybufs=3, ubufs=5, psA=4, psB=3,
)


_NC_CACHE = {}


def _build_bass(cfg=None):
    cfg = dict(CFG, **(cfg or {}))
    key = repr(sorted((k, repr(v)) for k, v in cfg.items()))
    if key in _NC_CACHE:
        return _NC_CACHE[key]

    f32 = mybir.dt.float32
    bf16 = mybir.dt.bfloat16

    SKEW = cfg["skew"]
    in_blocks = list(cfg["in_blocks"])
    out_blocks = list(cfg["out_blocks"])
    assert sum(in_blocks) == NG and sum(out_blocks) == NG

    nc = bacc.Bacc("TRN2", target_bir_lowering=False, debug=False,
                   num_devices=N_CORES)
    xt_d = nc.dram_tensor("xt", [128, FREE], bf16, kind="ExternalInput")
    W1_d = nc.dram_tensor("W1", [128, 128], bf16, kind="ExternalInput")
    W2_d = nc.dram_tensor("W2", [128, 128], bf16, kind="ExternalInput")
    Y_d = nc.dram_tensor("Y", [128, FREE], bf16, kind="ExternalOutput")

    # group -> (in-block index, group offset within block)
    g2in = {}
    off = 0
    for bi, nb in enumerate(in_blocks):
        for gi in range(nb):
            g2in[off + gi] = (bi, gi)
        off += nb
    in_starts = np.cumsum([0] + in_blocks[:-1])
    g2out = {}
    off = 0
    for bi, nb in enumerate(out_blocks):
        for gi in range(nb):
            g2out[off + gi] = (bi, gi)
        off += nb
    out_starts = np.cumsum([0] + out_blocks[:-1])

    with tile.TileContext(nc) as tc:
        with (
            tc.tile_pool(name="wpool", bufs=1) as wpool,
            tc.tile_pool(name="xpool", bufs=cfg["xbufs"]) as xpool,
            tc.tile_pool(name="upool", bufs=cfg["ubufs"]) as upool,
            tc.tile_pool(name="ypool", bufs=cfg["ybufs"]) as ypool,
            tc.tile_pool(name="psA", bufs=cfg["psA"], space="PSUM") as psA,
            tc.tile_pool(name="psB", bufs=cfg["psB"], space="PSUM") as psB,
        ):
            W1_sb = wpool.tile([128, 128], bf16)
            nc.sync.dma_start(W1_sb[:], W1_d[:])
            W2_sb = wpool.tile([128, 128], bf16)
            nc.sync.dma_start(W2_sb[:], W2_d[:])

            def eng_list(names):
                return [getattr(nc, nm.strip()) for nm in names.split(",")]

            ucopy_engs = eng_list(cfg["ucopy_engs"])
            ycopy_engs = eng_list(cfg["ycopy_engs"])
            in_engs = eng_list(cfg["in_engs"])
            out_engs = eng_list(cfg["out_engs"])

            def copy(engs, i, dst, src):
                e = engs[i % len(engs)]
                if e is nc.scalar:
                    nc.scalar.copy(dst, src)
                else:
                    e.tensor_copy(dst, src)

            xg_tiles = {}
            yb_tiles = {}
            pu_tiles = {}
            u_tiles = {}

            def emit_A(g):
                bi, gi = g2in[g]
                if gi == 0:
                    nb = in_blocks[bi]
                    xg = xpool.tile([128, nb * 512], bf16, name="xg")
                    lo = int(in_starts[bi]) * 512
                    in_engs[bi % len(in_engs)].dma_start(
                        xg[:], xt_d[:, lo:lo + nb * 512])
                    xg_tiles[bi] = xg
                xg = xg_tiles[bi]
                pu = psA.tile([128, 512], f32, name="pu")
                for j in range(4):
                    c = (gi * 4 + j) * 128
                    nc.tensor.matmul(pu[:, j * 128:(j + 1) * 128],
                                     xg[:, c:c + 128], W1_sb[:],
                                     start=True, stop=True)
                u_sb = upool.tile([128, 512], bf16, name="u_sb")
                copy(ucopy_engs, g, u_sb[:], pu[:])
                pu_tiles[g] = pu
                u_tiles[g] = u_sb

            def emit_B(g):
                bi, gi = g2out[g]
                if gi == 0:
                    yb_tiles[bi] = ypool.tile([128, out_blocks[bi] * 512],
                                              bf16, name="yb")
                yb = yb_tiles[bi]
                py = psB.tile([128, 512], f32, name="py")
                nc.tensor.matmul(py[:], W2_sb[:], u_tiles.pop(g)[:],
                                 start=True, stop=True)
                pu_tiles.pop(g, None)
                copy(ycopy_engs, g, yb[:, gi * 512:(gi + 1) * 512], py[:])
                if gi == out_blocks[bi] - 1:
                    lo = int(out_starts[bi]) * 512
                    out_engs[bi % len(out_engs)].dma_start(
                        Y_d[:, lo:lo + out_blocks[bi] * 512], yb[:])

            for g in range(NG):
                emit_A(g)
                if g >= SKEW:
                    emit_B(g - SKEW)
            for g in range(NG - SKEW, NG):
                emit_B(g)

    nc.compile()
    _NC_CACHE[key] = nc
    return nc


def _prep_inputs(x, H, cfg=None):
    Hf = np.asarray(H, dtype=np.float32)
    W1 = (Hf[::32, ::32] * 8.0).astype(ml_dtypes.bfloat16)
    W2 = np.kron(np.eye(4, dtype=np.float32),
                 Hf[:32, :32] * 8.0).astype(ml_dtypes.bfloat16)
    xf = np.asarray(x, dtype=np.float32).reshape(R_TOTAL, DIM)
    in_maps = []
    for i in range(N_CORES):
        shard = xf[i * R:(i + 1) * R]                    # (R, DIM)
        xt = shard.reshape(R, 128, 32).transpose(1, 0, 2)
        xt = np.ascontiguousarray(xt, dtype=ml_dtypes.bfloat16)
        in_maps.append({"xt": xt.reshape(128, FREE), "W1": W1, "W2": W2})
    return in_maps


def _unscramble(results, cfg=None):
    outs = []
    for i in range(N_CORES):
        Y = np.asarray(results[i]["Y"], dtype=np.float32)   # [128, FREE]
        # Y[(rl,EA), 512g + 128j + EP] = y[16g+4j+rl, 32*EP+EA]
        y = Y.reshape(4, 32, NG, 4, 128).transpose(2, 3, 0, 4, 1)
        outs.append(y.reshape(R, DIM))
    return np.concatenate(outs, axis=0).reshape(4, 4096, DIM).astype(np.float32)


def kernel(x, H, _trace=False, _cfg=None):
    nc = _build_bass(_cfg)
    in_maps = _prep_inputs(x, H, _cfg)
    res = run_bass_kernel_spmd(nc, in_maps, core_ids=list(range(N_CORES)),
                               trace=_trace)
    out = _unscramble(res.results, _cfg)
    if _trace:
        return out, res
    return out


# revision 5
# speedup vs baseline: 2.9777x; 1.0697x over previous
"""Trainium2 kernel for nn_HadamardRotation: y = x @ H, H = 4096x4096 Walsh-Hadamard.

Strategy (v2: turn-free, data-stationary stage A)
-------------------------------------------------
H4096 = H128 (x) H32 (Kronecker over bit positions, d = 32*P + A):

    y[r, 32*EP + EA] = sum_{P,A} x[r, 32*P + A] * H128[P,EP] * H32[A,EA] / 64

Stage A loads the DATA as the stationary operand (lhsT), so the output
partition axis becomes the (row, A) free-chunk of x -- i.e. the stage-B
contraction bits (A) land on partitions with NO corner turn:

  xt DRAM [128, R*32] bf16:  xt[P, 32*r + A] = x[r, 32*P + A]   (host permute)
  MM_A (per 4 rows):  lhsT = xt chunk [128=(P), 128=(rl,A)], rhs = W1=H128/8
                      -> PSUM [(rl,A), EP];  4 chunks fill one [128,512] bank
  copy PSUM->SBUF bf16 (rotating scalar/vector/gpsimd)
  MM_B (per 16 rows): lhsT = W2 = I4 (x) H32/8 (contracts A, passes rl),
                      rhs = u [128,512] -> PSUM [(rl,EA), (j,EP)] = y
  copy -> staging, large contiguous out-DMAs.

DMA traffic is only 16.8MB in + 16.8MB out per core (HBM floor ~94us);
no SBUF->SBUF turn traffic at all.  Host does layout permutes + casts
(not timed), weights are exact (+-1/8) in bf16.

Data parallel over 8 cores: rows sharded 16384 -> 8 x 2048.
"""

import math
import numpy as np
import ml_dtypes

import concourse.bass as bass
import concourse.mybir as mybir
import concourse.tile as tile
from concourse import bacc
from concourse.bass_utils import run_bass_kernel_spmd

N_CORES = 8
DIM = 4096
R_TOTAL = 4 * 4096
R = R_TOTAL // N_CORES      # rows per core (2048)
NG = R // 16                # 16-row groups per core (128)
FREE = R * 32               # free extent of xt / Y (65536)
MODE = "bf16"

CFG = dict(
    skew=2,                       # PAIRS of groups emitted ahead of MM_B
    in_blocks=[2, 2, 2, 2] + [8] * 7,     # group-PAIRS per input DMA
    out_blocks=[8] * 6 + [4, 4, 2, 2, 1, 1, 1, 1],  # group-PAIRS per out DMA
    ucopy_engs="scalar",
    ycopy_engs="vector,vector,vector,vector,vector,vector,vector,vector,vector,scalar",
    in_engs="sync",
    out_engs="gpsimd",
    xbufs=3, ybufs=3, ubufs=4, psA=2, psB=2,
)


_NC_CACHE = {}


def _build_bass(cfg=None):
    cfg = dict(CFG, **(cfg or {}))
    key = repr(sorted((k, repr(v)) for k, v in cfg.items()))
    if key in _NC_CACHE:
        return _NC_CACHE[key]

    f32 = mybir.dt.float32
    bf16 = mybir.dt.bfloat16

    SKEW = cfg["skew"]
    NP = NG // 2                  # group-pairs per core
    in_blocks = list(cfg["in_blocks"])
    out_blocks = list(cfg["out_blocks"])
    assert sum(in_blocks) == NP and sum(out_blocks) == NP

    nc = bacc.Bacc("TRN2", target_bir_lowering=False, debug=False,
                   num_devices=N_CORES)
    xt_d = nc.dram_tensor("xt", [128, FREE], bf16, kind="ExternalInput")
    W1_d = nc.dram_tensor("W1", [128, 128], bf16, kind="ExternalInput")
    W2_d = nc.dram_tensor("W2", [128, 128], bf16, kind="ExternalInput")
    Y_d = nc.dram_tensor("Y", [128, FREE], bf16, kind="ExternalOutput")

    # group -> (in-block index, group offset within block)
    g2in = {}
    off = 0
    for bi, nb in enumerate(in_blocks):
        for gi in range(nb):
            g2in[off + gi] = (bi, gi)
        off += nb
    in_starts = np.cumsum([0] + in_blocks[:-1])
    g2out = {}
    off = 0
    for bi, nb in enumerate(out_blocks):
        for gi in range(nb):
            g2out[off + gi] = (bi, gi)
        off += nb
    out_starts = np.cumsum([0] + out_blocks[:-1])

    with tile.TileContext(nc) as tc:
        with (
            tc.tile_pool(name="wpool", bufs=1) as wpool,
            tc.tile_pool(name="xpool", bufs=cfg["xbufs"]) as xpool,
            tc.tile_pool(name="upool", bufs=cfg["ubufs"]) as upool,
            tc.tile_pool(name="ypool", bufs=cfg["ybufs"]) as ypool,
            tc.tile_pool(name="psA", bufs=cfg["psA"], space="PSUM") as psA,
            tc.tile_pool(name="psB", bufs=cfg["psB"], space="PSUM") as psB,
        ):
            W1_sb = wpool.tile([128, 128], bf16)
            nc.sync.dma_start(W1_sb[:], W1_d[:])
            W2_sb = wpool.tile([128, 128], bf16)
            nc.sync.dma_start(W2_sb[:], W2_d[:])

            def eng_list(names):
                return [getattr(nc, nm.strip()) for nm in names.split(",")]

            ucopy_engs = eng_list(cfg["ucopy_engs"])
            ycopy_engs = eng_list(cfg["ycopy_engs"])
            in_engs = eng_list(cfg["in_engs"])
            out_engs = eng_list(cfg["out_engs"])

            def copy(engs, i, dst, src):
                e = engs[i % len(engs)]
                if e is nc.scalar:
                    nc.scalar.copy(dst, src)
                else:
                    e.tensor_copy(dst, src)

            xg_tiles = {}
            yb_tiles = {}
            pu_tiles = {}
            u_tiles = {}

            def emit_A(gp):
                bi, gi = g2in[gp]
                if gi == 0:
                    nb = in_blocks[bi]
                    xg = xpool.tile([128, nb * 1024], bf16, name="xg")
                    lo = int(in_starts[bi]) * 1024
                    in_engs[bi % len(in_engs)].dma_start(
                        xg[:], xt_d[:, lo:lo + nb * 1024])
                    xg_tiles[bi] = xg
                xg = xg_tiles[bi]
                pu = psA.tile([128, 1024], f32, name="pu")
                for j in range(8):
                    c = (gi * 8 + j) * 128
                    nc.tensor.matmul(pu[:, j * 128:(j + 1) * 128],
                                     xg[:, c:c + 128], W1_sb[:],
                                     start=True, stop=True)
                u_sb = upool.tile([128, 1024], bf16, name="u_sb")
                copy(ucopy_engs, gp, u_sb[:], pu[:])
                pu_tiles[gp] = pu
                u_tiles[gp] = u_sb

            def emit_B(gp):
                bi, gi = g2out[gp]
                if gi == 0:
                    yb_tiles[bi] = ypool.tile([128, out_blocks[bi] * 1024],
                                              bf16, name="yb")
                yb = yb_tiles[bi]
                py = psB.tile([128, 1024], f32, name="py")
                u_sb = u_tiles.pop(gp)
                nc.tensor.matmul(py[:, :512], W2_sb[:], u_sb[:, :512],
                                 start=True, stop=True)
                nc.tensor.matmul(py[:, 512:], W2_sb[:], u_sb[:, 512:],
                                 start=True, stop=True)
                pu_tiles.pop(gp, None)
                copy(ycopy_engs, gp, yb[:, gi * 1024:(gi + 1) * 1024], py[:])
                if gi == out_blocks[bi] - 1:
                    lo = int(out_starts[bi]) * 1024
                    out_engs[bi % len(out_engs)].dma_start(
                        Y_d[:, lo:lo + out_blocks[bi] * 1024], yb[:])

            for gp in range(NP):
                emit_A(gp)
                if gp >= SKEW:
                    emit_B(gp - SKEW)
            for gp in range(NP - SKEW, NP):
                emit_B(gp)

    nc.compile()
    _NC_CACHE[key] = nc
    return nc


def _prep_inputs(x, H, cfg=None):
    Hf = np.asarray(H, dtype=np.float32)
    W1 = (Hf[::32, ::32] * 8.0).astype(ml_dtypes.bfloat16)
    W2 = np.kron(np.eye(4, dtype=np.float32),
                 Hf[:32, :32] * 8.0).astype(ml_dtypes.bfloat16)
    xf = np.asarray(x, dtype=np.float32).reshape(R_TOTAL, DIM)
    in_maps = []
    for i in range(N_CORES):
        shard = xf[i * R:(i + 1) * R]                    # (R, DIM)
        xt = shard.reshape(R, 128, 32).transpose(1, 0, 2)
        xt = np.ascontiguousarray(xt, dtype=ml_dtypes.bfloat16)
        in_maps.append({"xt": xt.reshape(128, FREE), "W1": W1, "W2": W2})
    return in_maps


def _unscramble(results, cfg=None):
    outs = []
    for i in range(N_CORES):
        Y = np.asarray(results[i]["Y"], dtype=np.float32)   # [128, FREE]
        # Y[(rl,EA), 512g + 128j + EP] = y[16g+4j+rl, 32*EP+EA]
        y = Y.reshape(4, 32, NG, 4, 128).transpose(2, 3, 0, 4, 1)
        outs.append(y.reshape(R, DIM))
    return np.concatenate(outs, axis=0).reshape(4, 4096, DIM).astype(np.float32)


def kernel(x, H, _trace=False, _cfg=None):
    nc = _build_bass(_cfg)
    in_maps = _prep_inputs(x, H, _cfg)
    res = run_bass_kernel_spmd(nc, in_maps, core_ids=list(range(N_CORES)),
                               trace=_trace)
    out = _unscramble(res.results, _cfg)
    if _trace:
        return out, res
    return out


# revision 7
# speedup vs baseline: 3.0470x; 1.0233x over previous
"""Trainium2 kernel for nn_HadamardRotation: y = x @ H, H = 4096x4096 Walsh-Hadamard.

Strategy (v2: turn-free, data-stationary stage A)
-------------------------------------------------
H4096 = H128 (x) H32 (Kronecker over bit positions, d = 32*P + A):

    y[r, 32*EP + EA] = sum_{P,A} x[r, 32*P + A] * H128[P,EP] * H32[A,EA] / 64

Stage A loads the DATA as the stationary operand (lhsT), so the output
partition axis becomes the (row, A) free-chunk of x -- i.e. the stage-B
contraction bits (A) land on partitions with NO corner turn:

  xt DRAM [128, R*32] bf16:  xt[P, 32*r + A] = x[r, 32*P + A]   (host permute)
  MM_A (per 4 rows):  lhsT = xt chunk [128=(P), 128=(rl,A)], rhs = W1=H128/8
                      -> PSUM [(rl,A), EP];  4 chunks fill one [128,512] bank
  copy PSUM->SBUF bf16 (rotating scalar/vector/gpsimd)
  MM_B (per 16 rows): lhsT = W2 = I4 (x) H32/8 (contracts A, passes rl),
                      rhs = u [128,512] -> PSUM [(rl,EA), (j,EP)] = y
  copy -> staging, large contiguous out-DMAs.

DMA traffic is only 16.8MB in + 16.8MB out per core (HBM floor ~94us);
no SBUF->SBUF turn traffic at all.  Host does layout permutes + casts
(not timed), weights are exact (+-1/8) in bf16.

Data parallel over 8 cores: rows sharded 16384 -> 8 x 2048.
"""

import math
import numpy as np
import ml_dtypes

import concourse.bass as bass
import concourse.mybir as mybir
import concourse.tile as tile
from concourse import bacc
from concourse.bass_utils import run_bass_kernel_spmd

N_CORES = 8
DIM = 4096
R_TOTAL = 4 * 4096
R = R_TOTAL // N_CORES      # rows per core (2048)
NG = R // 16                # 16-row groups per core (128)
FREE = R * 32               # free extent of xt / Y (65536)
MODE = "bf16"

CFG = dict(
    skew=2,                       # PAIRS of groups emitted ahead of MM_B
    in_blocks=[1, 1, 2, 4] + [8] * 7,     # group-PAIRS per input DMA
    out_blocks=[8] * 6 + [4, 4, 3, 2, 1, 1, 1],     # group-PAIRS per out DMA
    ucopy_engs="scalar",
    ycopy_engs="vector,vector,vector,vector,vector,vector,vector,vector,vector,scalar",
    in_engs="sync",
    out_engs="gpsimd",
    xbufs=4, ybufs=4, ubufs=6, psA=2, psB=2,
)


_NC_CACHE = {}


def _build_bass(cfg=None):
    cfg = dict(CFG, **(cfg or {}))
    key = repr(sorted((k, repr(v)) for k, v in cfg.items()))
    if key in _NC_CACHE:
        return _NC_CACHE[key]

    f32 = mybir.dt.float32
    bf16 = mybir.dt.bfloat16

    SKEW = cfg["skew"]
    NP = NG // 2                  # group-pairs per core
    in_blocks = list(cfg["in_blocks"])
    out_blocks = list(cfg["out_blocks"])
    assert sum(in_blocks) == NP and sum(out_blocks) == NP

    nc = bacc.Bacc("TRN2", target_bir_lowering=False, debug=False,
                   num_devices=N_CORES)
    xt_d = nc.dram_tensor("xt", [128, FREE], bf16, kind="ExternalInput")
    W1_d = nc.dram_tensor("W1", [128, 128], bf16, kind="ExternalInput")
    W2_d = nc.dram_tensor("W2", [128, 128], bf16, kind="ExternalInput")
    Y_d = nc.dram_tensor("Y", [128, FREE], bf16, kind="ExternalOutput")

    # group -> (in-block index, group offset within block)
    g2in = {}
    off = 0
    for bi, nb in enumerate(in_blocks):
        for gi in range(nb):
            g2in[off + gi] = (bi, gi)
        off += nb
    in_starts = np.cumsum([0] + in_blocks[:-1])
    g2out = {}
    off = 0
    for bi, nb in enumerate(out_blocks):
        for gi in range(nb):
            g2out[off + gi] = (bi, gi)
        off += nb
    out_starts = np.cumsum([0] + out_blocks[:-1])

    with tile.TileContext(nc) as tc:
        with (
            tc.tile_pool(name="wpool", bufs=1) as wpool,
            tc.tile_pool(name="xpool", bufs=cfg["xbufs"]) as xpool,
            tc.tile_pool(name="upool", bufs=cfg["ubufs"]) as upool,
            tc.tile_pool(name="ypool", bufs=cfg["ybufs"]) as ypool,
            tc.tile_pool(name="psA", bufs=cfg["psA"], space="PSUM") as psA,
            tc.tile_pool(name="psB", bufs=cfg["psB"], space="PSUM") as psB,
        ):
            W1_sb = wpool.tile([128, 128], bf16)
            nc.gpsimd.dma_start(W1_sb[:], W1_d[:])
            W2_sb = wpool.tile([128, 128], bf16)
            nc.gpsimd.dma_start(W2_sb[:], W2_d[:])

            def eng_list(names):
                return [getattr(nc, nm.strip()) for nm in names.split(",")]

            ucopy_engs = eng_list(cfg["ucopy_engs"])
            ycopy_engs = eng_list(cfg["ycopy_engs"])
            in_engs = eng_list(cfg["in_engs"])
            out_engs = eng_list(cfg["out_engs"])

            def copy(engs, i, dst, src):
                e = engs[i % len(engs)]
                if e is nc.scalar:
                    nc.scalar.copy(dst, src)
                else:
                    e.tensor_copy(dst, src)

            xg_tiles = {}
            yb_tiles = {}
            pu_tiles = {}
            u_tiles = {}

            def emit_A(gp):
                bi, gi = g2in[gp]
                if gi == 0:
                    nb = in_blocks[bi]
                    xg = xpool.tile([128, nb * 1024], bf16, name="xg")
                    lo = int(in_starts[bi]) * 1024
                    in_engs[bi % len(in_engs)].dma_start(
                        xg[:], xt_d[:, lo:lo + nb * 1024])
                    xg_tiles[bi] = xg
                xg = xg_tiles[bi]
                pu = psA.tile([128, 1024], f32, name="pu")
                for j in range(8):
                    c = (gi * 8 + j) * 128
                    nc.tensor.matmul(pu[:, j * 128:(j + 1) * 128],
                                     xg[:, c:c + 128], W1_sb[:],
                                     start=True, stop=True)
                u_sb = upool.tile([128, 1024], bf16, name="u_sb")
                copy(ucopy_engs, gp, u_sb[:], pu[:])
                pu_tiles[gp] = pu
                u_tiles[gp] = u_sb

            def emit_B(gp):
                bi, gi = g2out[gp]
                if gi == 0:
                    yb_tiles[bi] = ypool.tile([128, out_blocks[bi] * 1024],
                                              bf16, name="yb")
                yb = yb_tiles[bi]
                py = psB.tile([128, 1024], f32, name="py")
                u_sb = u_tiles.pop(gp)
                nc.tensor.matmul(py[:, :512], W2_sb[:], u_sb[:, :512],
                                 start=True, stop=True)
                nc.tensor.matmul(py[:, 512:], W2_sb[:], u_sb[:, 512:],
                                 start=True, stop=True)
                pu_tiles.pop(gp, None)
                copy(ycopy_engs, gp, yb[:, gi * 1024:(gi + 1) * 1024], py[:])
                if gi == out_blocks[bi] - 1:
                    lo = int(out_starts[bi]) * 1024
                    out_engs[bi % len(out_engs)].dma_start(
                        Y_d[:, lo:lo + out_blocks[bi] * 1024], yb[:])

            for gp in range(NP):
                emit_A(gp)
                if gp >= SKEW:
                    emit_B(gp - SKEW)
            for gp in range(NP - SKEW, NP):
                emit_B(gp)

    nc.compile()
    _NC_CACHE[key] = nc
    return nc


def _prep_inputs(x, H, cfg=None):
    Hf = np.asarray(H, dtype=np.float32)
    W1 = (Hf[::32, ::32] * 8.0).astype(ml_dtypes.bfloat16)
    W2 = np.kron(np.eye(4, dtype=np.float32),
                 Hf[:32, :32] * 8.0).astype(ml_dtypes.bfloat16)
    xf = np.asarray(x, dtype=np.float32).reshape(R_TOTAL, DIM)
    in_maps = []
    for i in range(N_CORES):
        shard = xf[i * R:(i + 1) * R]                    # (R, DIM)
        xt = shard.reshape(R, 128, 32).transpose(1, 0, 2)
        xt = np.ascontiguousarray(xt, dtype=ml_dtypes.bfloat16)
        in_maps.append({"xt": xt.reshape(128, FREE), "W1": W1, "W2": W2})
    return in_maps


def _unscramble(results, cfg=None):
    outs = []
    for i in range(N_CORES):
        Y = np.asarray(results[i]["Y"], dtype=np.float32)   # [128, FREE]
        # Y[(rl,EA), 512g + 128j + EP] = y[16g+4j+rl, 32*EP+EA]
        y = Y.reshape(4, 32, NG, 4, 128).transpose(2, 3, 0, 4, 1)
        outs.append(y.reshape(R, DIM))
    return np.concatenate(outs, axis=0).reshape(4, 4096, DIM).astype(np.float32)


def kernel(x, H, _trace=False, _cfg=None):
    nc = _build_bass(_cfg)
    in_maps = _prep_inputs(x, H, _cfg)
    res = run_bass_kernel_spmd(nc, in_maps, core_ids=list(range(N_CORES)),
                               trace=_trace)
    out = _unscramble(res.results, _cfg)
    if _trace:
        return out, res
    return out


# revision 8
# speedup vs baseline: 3.1074x; 1.0198x over previous
"""Trainium2 kernel for nn_HadamardRotation: y = x @ H, H = 4096x4096 Walsh-Hadamard.

Strategy (v2: turn-free, data-stationary stage A)
-------------------------------------------------
H4096 = H128 (x) H32 (Kronecker over bit positions, d = 32*P + A):

    y[r, 32*EP + EA] = sum_{P,A} x[r, 32*P + A] * H128[P,EP] * H32[A,EA] / 64

Stage A loads the DATA as the stationary operand (lhsT), so the output
partition axis becomes the (row, A) free-chunk of x -- i.e. the stage-B
contraction bits (A) land on partitions with NO corner turn:

  xt DRAM [128, R*32] bf16:  xt[P, 32*r + A] = x[r, 32*P + A]   (host permute)
  MM_A (per 4 rows):  lhsT = xt chunk [128=(P), 128=(rl,A)], rhs = W1=H128/8
                      -> PSUM [(rl,A), EP];  4 chunks fill one [128,512] bank
  copy PSUM->SBUF bf16 (rotating scalar/vector/gpsimd)
  MM_B (per 16 rows): lhsT = W2 = I4 (x) H32/8 (contracts A, passes rl),
                      rhs = u [128,512] -> PSUM [(rl,EA), (j,EP)] = y
  copy -> staging, large contiguous out-DMAs.

DMA traffic is only 16.8MB in + 16.8MB out per core (HBM floor ~94us);
no SBUF->SBUF turn traffic at all.  Host does layout permutes + casts
(not timed), weights are exact (+-1/8) in bf16.

Data parallel over 8 cores: rows sharded 16384 -> 8 x 2048.
"""

import math
import numpy as np
import ml_dtypes

import concourse.bass as bass
import concourse.mybir as mybir
import concourse.tile as tile
from concourse import bacc
from concourse.bass_utils import run_bass_kernel_spmd

N_CORES = 8
DIM = 4096
R_TOTAL = 4 * 4096
R = R_TOTAL // N_CORES      # rows per core (2048)
NG = R // 16                # 16-row groups per core (128)
FREE = R * 32               # free extent of xt / Y (65536)
MODE = "bf16"

CFG = dict(
    skew=2,                       # PAIRS of groups emitted ahead of MM_B
    in_blocks=[1, 1, 2, 4] + [8] * 7,     # group-PAIRS per input DMA
    out_blocks=[8] * 6 + [4, 4, 3, 2, 1, 1, 1],     # group-PAIRS per out DMA
    ucopy_engs="scalar",
    ycopy_engs="vector,vector,vector,vector,vector,vector,vector,vector,vector,scalar",
    in_engs="sync",
    out_engs="gpsimd,sync",
    xbufs=4, ybufs=5, ubufs=6, psA=2, psB=2,
)


_NC_CACHE = {}


def _build_bass(cfg=None):
    cfg = dict(CFG, **(cfg or {}))
    key = repr(sorted((k, repr(v)) for k, v in cfg.items()))
    if key in _NC_CACHE:
        return _NC_CACHE[key]

    f32 = mybir.dt.float32
    bf16 = mybir.dt.bfloat16

    SKEW = cfg["skew"]
    NP = NG // 2                  # group-pairs per core
    in_blocks = list(cfg["in_blocks"])
    out_blocks = list(cfg["out_blocks"])
    assert sum(in_blocks) == NP and sum(out_blocks) == NP

    nc = bacc.Bacc("TRN2", target_bir_lowering=False, debug=False,
                   num_devices=N_CORES)
    xt_d = nc.dram_tensor("xt", [128, FREE], bf16, kind="ExternalInput")
    W1_d = nc.dram_tensor("W1", [128, 128], bf16, kind="ExternalInput")
    W2_d = nc.dram_tensor("W2", [128, 128], bf16, kind="ExternalInput")
    Y_d = nc.dram_tensor("Y", [128, FREE], bf16, kind="ExternalOutput")

    # group -> (in-block index, group offset within block)
    g2in = {}
    off = 0
    for bi, nb in enumerate(in_blocks):
        for gi in range(nb):
            g2in[off + gi] = (bi, gi)
        off += nb
    in_starts = np.cumsum([0] + in_blocks[:-1])
    g2out = {}
    off = 0
    for bi, nb in enumerate(out_blocks):
        for gi in range(nb):
            g2out[off + gi] = (bi, gi)
        off += nb
    out_starts = np.cumsum([0] + out_blocks[:-1])

    with tile.TileContext(nc) as tc:
        with (
            tc.tile_pool(name="wpool", bufs=1) as wpool,
            tc.tile_pool(name="xpool", bufs=cfg["xbufs"]) as xpool,
            tc.tile_pool(name="upool", bufs=cfg["ubufs"]) as upool,
            tc.tile_pool(name="ypool", bufs=cfg["ybufs"]) as ypool,
            tc.tile_pool(name="psA", bufs=cfg["psA"], space="PSUM") as psA,
            tc.tile_pool(name="psB", bufs=cfg["psB"], space="PSUM") as psB,
        ):
            W1_sb = wpool.tile([128, 128], bf16)
            nc.gpsimd.dma_start(W1_sb[:], W1_d[:])
            W2_sb = wpool.tile([128, 128], bf16)
            nc.gpsimd.dma_start(W2_sb[:], W2_d[:])

            def eng_list(names):
                return [getattr(nc, nm.strip()) for nm in names.split(",")]

            ucopy_engs = eng_list(cfg["ucopy_engs"])
            ycopy_engs = eng_list(cfg["ycopy_engs"])
            in_engs = eng_list(cfg["in_engs"])
            out_engs = eng_list(cfg["out_engs"])

            def copy(engs, i, dst, src):
                e = engs[i % len(engs)]
                if e is nc.scalar:
                    nc.scalar.copy(dst, src)
                else:
                    e.tensor_copy(dst, src)

            xg_tiles = {}
            yb_tiles = {}
            pu_tiles = {}
            u_tiles = {}

            def emit_A(gp):
                bi, gi = g2in[gp]
                if gi == 0:
                    nb = in_blocks[bi]
                    xg = xpool.tile([128, nb * 1024], bf16, name="xg")
                    lo = int(in_starts[bi]) * 1024
                    in_engs[bi % len(in_engs)].dma_start(
                        xg[:], xt_d[:, lo:lo + nb * 1024])
                    xg_tiles[bi] = xg
                xg = xg_tiles[bi]
                pu = psA.tile([128, 1024], f32, name="pu")
                for j in range(8):
                    c = (gi * 8 + j) * 128
                    nc.tensor.matmul(pu[:, j * 128:(j + 1) * 128],
                                     xg[:, c:c + 128], W1_sb[:],
                                     start=True, stop=True)
                u_sb = upool.tile([128, 1024], bf16, name="u_sb")
                copy(ucopy_engs, gp, u_sb[:], pu[:])
                pu_tiles[gp] = pu
                u_tiles[gp] = u_sb

            def emit_B(gp):
                bi, gi = g2out[gp]
                if gi == 0:
                    yb_tiles[bi] = ypool.tile([128, out_blocks[bi] * 1024],
                                              bf16, name="yb")
                yb = yb_tiles[bi]
                py = psB.tile([128, 1024], f32, name="py")
                u_sb = u_tiles.pop(gp)
                nc.tensor.matmul(py[:, :512], W2_sb[:], u_sb[:, :512],
                                 start=True, stop=True)
                nc.tensor.matmul(py[:, 512:], W2_sb[:], u_sb[:, 512:],
                                 start=True, stop=True)
                pu_tiles.pop(gp, None)
                copy(ycopy_engs, gp, yb[:, gi * 1024:(gi + 1) * 1024], py[:])
                if gi == out_blocks[bi] - 1:
                    lo = int(out_starts[bi]) * 1024
                    out_engs[bi % len(out_engs)].dma_start(
                        Y_d[:, lo:lo + out_blocks[bi] * 1024], yb[:])

            for gp in range(NP):
                emit_A(gp)
                if gp >= SKEW:
                    emit_B(gp - SKEW)
            for gp in range(NP - SKEW, NP):
                emit_B(gp)

    nc.compile()
    _NC_CACHE[key] = nc
    return nc


def _prep_inputs(x, H, cfg=None):
    Hf = np.asarray(H, dtype=np.float32)
    W1 = (Hf[::32, ::32] * 8.0).astype(ml_dtypes.bfloat16)
    W2 = np.kron(np.eye(4, dtype=np.float32),
                 Hf[:32, :32] * 8.0).astype(ml_dtypes.bfloat16)
    xf = np.asarray(x, dtype=np.float32).reshape(R_TOTAL, DIM)
    in_maps = []
    for i in range(N_CORES):
        shard = xf[i * R:(i + 1) * R]                    # (R, DIM)
        xt = shard.reshape(R, 128, 32).transpose(1, 0, 2)
        xt = np.ascontiguousarray(xt, dtype=ml_dtypes.bfloat16)
        in_maps.append({"xt": xt.reshape(128, FREE), "W1": W1, "W2": W2})
    return in_maps


def _unscramble(results, cfg=None):
    outs = []
    for i in range(N_CORES):
        Y = np.asarray(results[i]["Y"], dtype=np.float32)   # [128, FREE]
        # Y[(rl,EA), 512g + 128j + EP] = y[16g+4j+rl, 32*EP+EA]
        y = Y.reshape(4, 32, NG, 4, 128).transpose(2, 3, 0, 4, 1)
        outs.append(y.reshape(R, DIM))
    return np.concatenate(outs, axis=0).reshape(4, 4096, DIM).astype(np.float32)


def kernel(x, H, _trace=False, _cfg=None):
    nc = _build_bass(_cfg)
    in_maps = _prep_inputs(x, H, _cfg)
    res = run_bass_kernel_spmd(nc, in_maps, core_ids=list(range(N_CORES)),
                               trace=_trace)
    out = _unscramble(res.results, _cfg)
    if _trace:
        return out, res
    return out


# revision 9
# speedup vs baseline: 3.2938x; 1.0600x over previous
"""Trainium2 kernel for nn_HadamardRotation: y = x @ H, H = 4096x4096 Walsh-Hadamard.

Strategy (v2: turn-free, data-stationary stage A)
-------------------------------------------------
H4096 = H128 (x) H32 (Kronecker over bit positions, d = 32*P + A):

    y[r, 32*EP + EA] = sum_{P,A} x[r, 32*P + A] * H128[P,EP] * H32[A,EA] / 64

Stage A loads the DATA as the stationary operand (lhsT), so the output
partition axis becomes the (row, A) free-chunk of x -- i.e. the stage-B
contraction bits (A) land on partitions with NO corner turn:

  xt DRAM [128, R*32] bf16:  xt[P, 32*r + A] = x[r, 32*P + A]   (host permute)
  MM_A (per 4 rows):  lhsT = xt chunk [128=(P), 128=(rl,A)], rhs = W1=H128/8
                      -> PSUM [(rl,A), EP];  4 chunks fill one [128,512] bank
  copy PSUM->SBUF bf16 (rotating scalar/vector/gpsimd)
  MM_B (per 16 rows): lhsT = W2 = I4 (x) H32/8 (contracts A, passes rl),
                      rhs = u [128,512] -> PSUM [(rl,EA), (j,EP)] = y
  copy -> staging, large contiguous out-DMAs.

DMA traffic is only 16.8MB in + 16.8MB out per core (HBM floor ~94us);
no SBUF->SBUF turn traffic at all.  Host does layout permutes + casts
(not timed), weights are exact (+-1/8) in bf16.

Data parallel over 8 cores: rows sharded 16384 -> 8 x 2048.
"""

import math
import numpy as np
import ml_dtypes

import concourse.bass as bass
import concourse.mybir as mybir
import concourse.tile as tile
from concourse import bacc
from concourse.bass_utils import run_bass_kernel_spmd

N_CORES = 8
DIM = 4096
R_TOTAL = 4 * 4096
R = R_TOTAL // N_CORES      # rows per core (2048)
NG = R // 16                # 16-row groups per core (128)
FREE = R * 32               # free extent of xt / Y (65536)
MODE = "bf16"

CFG = dict(
    skew=2,                       # PAIRS of groups emitted ahead of MM_B
    in_blocks=[1, 1, 2, 4] + [4] * 14,    # group-PAIRS per input DMA
    out_blocks=[8] * 6 + [4, 4, 3, 2, 1, 1, 1],     # group-PAIRS per out DMA
    ucopy_engs="scalar",
    ycopy_engs="vector,vector,vector,vector,vector,vector,vector,vector,vector,scalar",
    in_engs="sync",
    out_engs="gpsimd,sync",
    xbufs=8, ybufs=4, ubufs=6, psA=2, psB=2,
)


_NC_CACHE = {}


def _build_bass(cfg=None):
    cfg = dict(CFG, **(cfg or {}))
    key = repr(sorted((k, repr(v)) for k, v in cfg.items()))
    if key in _NC_CACHE:
        return _NC_CACHE[key]

    f32 = mybir.dt.float32
    bf16 = mybir.dt.bfloat16

    SKEW = cfg["skew"]
    NP = NG // 2                  # group-pairs per core
    in_blocks = list(cfg["in_blocks"])
    out_blocks = list(cfg["out_blocks"])
    assert sum(in_blocks) == NP and sum(out_blocks) == NP

    nc = bacc.Bacc("TRN2", target_bir_lowering=False, debug=False,
                   num_devices=N_CORES)
    xt_d = nc.dram_tensor("xt", [128, FREE], bf16, kind="ExternalInput")
    W1_d = nc.dram_tensor("W1", [128, 128], bf16, kind="ExternalInput")
    W2_d = nc.dram_tensor("W2", [128, 128], bf16, kind="ExternalInput")
    Y_d = nc.dram_tensor("Y", [128, FREE], bf16, kind="ExternalOutput")

    # group -> (in-block index, group offset within block)
    g2in = {}
    off = 0
    for bi, nb in enumerate(in_blocks):
        for gi in range(nb):
            g2in[off + gi] = (bi, gi)
        off += nb
    in_starts = np.cumsum([0] + in_blocks[:-1])
    g2out = {}
    off = 0
    for bi, nb in enumerate(out_blocks):
        for gi in range(nb):
            g2out[off + gi] = (bi, gi)
        off += nb
    out_starts = np.cumsum([0] + out_blocks[:-1])

    with tile.TileContext(nc) as tc:
        with (
            tc.tile_pool(name="wpool", bufs=1) as wpool,
            tc.tile_pool(name="xpool", bufs=cfg["xbufs"]) as xpool,
            tc.tile_pool(name="upool", bufs=cfg["ubufs"]) as upool,
            tc.tile_pool(name="ypool", bufs=cfg["ybufs"]) as ypool,
            tc.tile_pool(name="psA", bufs=cfg["psA"], space="PSUM") as psA,
            tc.tile_pool(name="psB", bufs=cfg["psB"], space="PSUM") as psB,
        ):
            W1_sb = wpool.tile([128, 128], bf16)
            nc.gpsimd.dma_start(W1_sb[:], W1_d[:])
            W2_sb = wpool.tile([128, 128], bf16)
            nc.gpsimd.dma_start(W2_sb[:], W2_d[:])

            def eng_list(names):
                return [getattr(nc, nm.strip()) for nm in names.split(",")]

            ucopy_engs = eng_list(cfg["ucopy_engs"])
            ycopy_engs = eng_list(cfg["ycopy_engs"])
            in_engs = eng_list(cfg["in_engs"])
            out_engs = eng_list(cfg["out_engs"])

            def copy(engs, i, dst, src):
                e = engs[i % len(engs)]
                if e is nc.scalar:
                    nc.scalar.copy(dst, src)
                else:
                    e.tensor_copy(dst, src)

            xg_tiles = {}
            yb_tiles = {}
            pu_tiles = {}
            u_tiles = {}

            def emit_A(gp):
                bi, gi = g2in[gp]
                if gi == 0:
                    nb = in_blocks[bi]
                    xg = xpool.tile([128, nb * 1024], bf16, name="xg")
                    lo = int(in_starts[bi]) * 1024
                    in_engs[bi % len(in_engs)].dma_start(
                        xg[:], xt_d[:, lo:lo + nb * 1024])
                    xg_tiles[bi] = xg
                xg = xg_tiles[bi]
                pu = psA.tile([128, 1024], f32, name="pu")
                for j in range(8):
                    c = (gi * 8 + j) * 128
                    nc.tensor.matmul(pu[:, j * 128:(j + 1) * 128],
                                     xg[:, c:c + 128], W1_sb[:],
                                     start=True, stop=True)
                u_sb = upool.tile([128, 1024], bf16, name="u_sb")
                copy(ucopy_engs, gp, u_sb[:], pu[:])
                pu_tiles[gp] = pu
                u_tiles[gp] = u_sb

            def emit_B(gp):
                bi, gi = g2out[gp]
                if gi == 0:
                    yb_tiles[bi] = ypool.tile([128, out_blocks[bi] * 1024],
                                              bf16, name="yb")
                yb = yb_tiles[bi]
                py = psB.tile([128, 1024], f32, name="py")
                u_sb = u_tiles.pop(gp)
                nc.tensor.matmul(py[:, :512], W2_sb[:], u_sb[:, :512],
                                 start=True, stop=True)
                nc.tensor.matmul(py[:, 512:], W2_sb[:], u_sb[:, 512:],
                                 start=True, stop=True)
                pu_tiles.pop(gp, None)
                copy(ycopy_engs, gp, yb[:, gi * 1024:(gi + 1) * 1024], py[:])
                if gi == out_blocks[bi] - 1:
                    lo = int(out_starts[bi]) * 1024
                    out_engs[bi % len(out_engs)].dma_start(
                        Y_d[:, lo:lo + out_blocks[bi] * 1024], yb[:])

            for gp in range(NP):
                emit_A(gp)
                if gp >= SKEW:
                    emit_B(gp - SKEW)
            for gp in range(NP - SKEW, NP):
                emit_B(gp)

    nc.compile()
    _NC_CACHE[key] = nc
    return nc


def _prep_inputs(x, H, cfg=None):
    Hf = np.asarray(H, dtype=np.float32)
    W1 = (Hf[::32, ::32] * 8.0).astype(ml_dtypes.bfloat16)
    W2 = np.kron(np.eye(4, dtype=np.float32),
                 Hf[:32, :32] * 8.0).astype(ml_dtypes.bfloat16)
    xf = np.asarray(x, dtype=np.float32).reshape(R_TOTAL, DIM)
    in_maps = []
    for i in range(N_CORES):
        shard = xf[i * R:(i + 1) * R]                    # (R, DIM)
        xt = shard.reshape(R, 128, 32).transpose(1, 0, 2)
        xt = np.ascontiguousarray(xt, dtype=ml_dtypes.bfloat16)
        in_maps.append({"xt": xt.reshape(128, FREE), "W1": W1, "W2": W2})
    return in_maps


def _unscramble(results, cfg=None):
    outs = []
    for i in range(N_CORES):
        Y = np.asarray(results[i]["Y"], dtype=np.float32)   # [128, FREE]
        # Y[(rl,EA), 512g + 128j + EP] = y[16g+4j+rl, 32*EP+EA]
        y = Y.reshape(4, 32, NG, 4, 128).transpose(2, 3, 0, 4, 1)
        outs.append(y.reshape(R, DIM))
    return np.concatenate(outs, axis=0).reshape(4, 4096, DIM).astype(np.float32)


def kernel(x, H, _trace=False, _cfg=None):
    nc = _build_bass(_cfg)
    in_maps = _prep_inputs(x, H, _cfg)
    res = run_bass_kernel_spmd(nc, in_maps, core_ids=list(range(N_CORES)),
                               trace=_trace)
    out = _unscramble(res.results, _cfg)
    if _trace:
        return out, res
    return out
